# revision 6
# baseline (speedup 1.0000x reference)
"""Trainium2 Bass kernel for the ActionableRGM rotation-scan model.

Math: reference computes
    theta = cumsum(x @ om^T, axis=L)          [B,L,M]
    w     = [w0_0, rot(theta_m) @ (a_m, b_m)] [B,L,D]   (w0 = S^-1 z0)
    out   = S @ w
Since cumsum commutes with the (linear) projection by om, we host-compute
cumx = cumsum(x, axis=1) ([B,L,2], tiny) and device-compute
    theta' = cumx @ (om/2pi)^T                 (in revolutions)
    s, c   = sin/cos(2pi theta')               (range-reduced on device)
    out    = c @ P^T + s @ Q^T + const         (P,Q fold a,b into S columns)
with P[i,m] = S[i,1+2m] a_m + S[i,2+2m] b_m, Q[i,m] = S[i,2+2m] a_m - S[i,1+2m] b_m,
const = S[:,0] w0_0 (added host-side during the gather).

Sharding: pure data-parallel, batches 4i..4i+3 on core i, no collectives.
Device matmuls run in bf16 (hi/lo split for the angle projection so theta'
keeps ~fp27 precision); accumulation is fp32 in PSUM.
"""
import math

import numpy as np
import ml_dtypes

import concourse.bacc as bacc
import concourse.mybir as mybir
from concourse.tile import TileContext
from concourse import bass_utils

B, L, D = 32, 2048, 513
M = (D - 1) // 2          # 256
NCORES = 8
BS = B // NCORES          # 4 batches per core
LG = 512                  # l-group width (per theta tile)
NG = BS * (L // LG)       # 16 l-groups per core
MAGIC = 12582912.0        # 1.5 * 2^23: fp32 round-to-nearest-int trick
TWO_PI = 2.0 * math.pi

F32 = mybir.dt.float32
BF16 = mybir.dt.bfloat16
ALU = mybir.AluOpType
ACTF = mybir.ActivationFunctionType


def _build():
    nc = bacc.Bacc("TRN2", target_bir_lowering=False, debug=False)

    cumx6_d = nc.dram_tensor("cumx6", [6, BS * L], BF16, kind="ExternalInput")
    th_lhsT_d = nc.dram_tensor("th_lhsT", [6, M], BF16, kind="ExternalInput")
    pqt_d = nc.dram_tensor("pqt", [128, 4 * D], BF16, kind="ExternalInput")
    out_d = nc.dram_tensor("out", [BS, L, D], F32, kind="ExternalOutput")

    with TileContext(nc) as tc:
        with (
            tc.tile_pool(name="const", bufs=1) as cpool,
            tc.tile_pool(name="work", bufs=2) as wpool,
            tc.tile_pool(name="cs", bufs=4) as cspool,
            tc.tile_pool(name="osb", bufs=3) as opool,
            tc.tile_pool(name="thp", bufs=2, space="PSUM") as thpool,
            tc.tile_pool(name="pa", bufs=3, space="PSUM") as papool,
            tc.tile_pool(name="pb", bufs=3, space="PSUM") as pbpool,
        ):
            cumx6 = cpool.tile([6, BS * L], BF16)
            nc.sync.dma_start(out=cumx6[:], in_=cumx6_d.ap())
            th_lhsT = cpool.tile([6, M], BF16)
            nc.sync.dma_start(out=th_lhsT[:], in_=th_lhsT_d.ap())
            pqt = cpool.tile([128, 4 * D], BF16)
            nc.sync.dma_start(out=pqt[:], in_=pqt_d.ap())

            for g in range(NG):
                b, l0 = divmod(g, L // LG)
                l0 *= LG
                cs = []  # [c0, c1, s0, s1] tiles, [m=128, l=512] bf16
                for mc in range(2):
                    th = thpool.tile([128, LG], F32)
                    nc.tensor.matmul(
                        th[:],
                        th_lhsT[:, mc * 128:(mc + 1) * 128],
                        cumx6[:, b * L + l0: b * L + l0 + LG],
                        start=True, stop=True,
                    )
                    k = wpool.tile([128, LG], F32, tag="k")
                    nc.vector.tensor_scalar(k[:], th[:], MAGIC, MAGIC, ALU.add, ALU.subtract)
                    r = wpool.tile([128, LG], F32, tag="r")
                    nc.vector.scalar_tensor_tensor(r[:], k[:], -1.0, th[:], ALU.mult, ALU.add)
                    rc = wpool.tile([128, LG], F32, tag="rc")
                    nc.vector.add_range_wrap(rc[:], r[:], shift=0.25, bound=0.5, period=1.0)
                    s_t = cspool.tile([128, LG], BF16, tag="s")
                    nc.scalar.activation(s_t[:], r[:], ACTF.Sin, scale=TWO_PI)
                    c_t = cspool.tile([128, LG], BF16, tag="c")
                    nc.scalar.activation(c_t[:], rc[:], ACTF.Sin, scale=TWO_PI)
                    cs.append((c_t, s_t))
                # K-chunk order must match pqt rows: [P m0, P m1, Q m0, Q m1]
                kchunks = [cs[0][0], cs[1][0], cs[0][1], cs[1][1]]

                o = opool.tile([128, 4 * D], F32)
                for lc in range(4):
                    pa = papool.tile([128, 256], F32)
                    pb = pbpool.tile([128, 257], F32)
                    for kc in range(4):
                        lhsT = kchunks[kc][:, lc * 128:(lc + 1) * 128]
                        nc.tensor.matmul(pa[:], lhsT, pqt[:, kc * D: kc * D + 256],
                                         start=(kc == 0), stop=(kc == 3))
                        nc.tensor.matmul(pb[:], lhsT, pqt[:, kc * D + 256: (kc + 1) * D],
                                         start=(kc == 0), stop=(kc == 3))
                    nc.any.tensor_copy(o[:, lc * D: lc * D + 256], pa[:])
                    nc.any.tensor_copy(o[:, lc * D + 256: (lc + 1) * D], pb[:])
                nc.sync.dma_start(
                    out=out_d.ap()[b, l0: l0 + LG, :].rearrange("(lc p) i -> p lc i", p=128),
                    in_=o[:].rearrange("p (lc i) -> p lc i", lc=4),
                )
    nc.compile()
    return nc


_NC = None


def _get_nc():
    global _NC
    if _NC is None:
        _NC = _build()
    return _NC


def _bf16(x):
    return np.asarray(x, dtype=np.float32).astype(ml_dtypes.bfloat16)


def _prep(x, S, om, z0):
    """Host-side prep: exact cumsum + tiny dense algebra; returns per-core
    input maps plus the additive constant."""
    x = np.asarray(x, dtype=np.float32)
    S = np.asarray(S, dtype=np.float32)
    om = np.asarray(om, dtype=np.float32)
    z0 = np.asarray(z0, dtype=np.float32)

    S_inv = np.linalg.inv(S)
    w0 = S_inv @ z0
    a, bb = w0[1::2], w0[2::2]
    # P/Q: out[:, i] = sum_m c_m P[i,m] + s_m Q[i,m] + const_i
    Scol = S[:, 1:]                      # [D, 2M]
    P = Scol[:, 0::2] * a + Scol[:, 1::2] * bb     # [D, M]
    Q = Scol[:, 1::2] * a - Scol[:, 0::2] * bb
    const = (S[:, 0] * w0[0]).astype(np.float32)   # [D]
    pqt = np.concatenate([P.T, Q.T], axis=0)       # [2M=512, D]
    # device layout: [128 partitions, (kc, i)] with kc = K-chunk index
    pqt_bf = np.ascontiguousarray(
        _bf16(pqt).reshape(4, 128, D).transpose(1, 0, 2).reshape(128, 4 * D)
    )

    omr = (om / TWO_PI).astype(np.float32)         # [M, 2] in revolutions
    oh = _bf16(omr).astype(np.float32)
    ol = _bf16(omr - oh).astype(np.float32)
    th_lhsT = np.stack(
        [oh[:, 0], oh[:, 1], oh[:, 0], oh[:, 1], ol[:, 0], ol[:, 1]], axis=0
    )                                              # [6, M]
    th_lhsT_bf = _bf16(th_lhsT)

    cumx = np.cumsum(x, axis=1)                    # [B, L, 2] fp32
    ch = _bf16(cumx).astype(np.float32)
    cl = _bf16(cumx - ch).astype(np.float32)
    # rhs rows pair with th_lhsT rows: [chx, chy, clx, cly, chx, chy]
    cumx6 = np.stack(
        [ch[..., 0], ch[..., 1], cl[..., 0], cl[..., 1], ch[..., 0], ch[..., 1]],
        axis=1,
    )                                              # [B, 6, L]
    cumx6_bf = _bf16(cumx6)

    in_maps = []
    for ci in range(NCORES):
        shard = cumx6_bf[ci * BS:(ci + 1) * BS]            # [BS, 6, L]
        shard = np.ascontiguousarray(
            shard.transpose(1, 0, 2).reshape(6, BS * L)    # [6, (b l)]
        )
        in_maps.append({
            "cumx6": shard,
            "th_lhsT": th_lhsT_bf,
            "pqt": pqt_bf,
        })
    return in_maps, const


def kernel(x, S, om, z0):
    nc = _get_nc()
    in_maps, const = _prep(x, S, om, z0)
    res = bass_utils.run_bass_kernel_spmd(nc, in_maps, core_ids=list(range(NCORES)))
    out = np.concatenate([res.results[i]["out"] for i in range(NCORES)], axis=0)
    out += const[None, None, :]
    z_n = np.ascontiguousarray(out[:, -1])
    return out, z_n


# revision 7
# speedup vs baseline: 1.4236x; 1.4236x over previous
"""Trainium2 Bass kernel for the ActionableRGM rotation-scan model.

Math: reference computes
    theta = cumsum(x @ om^T, axis=L)          [B,L,M]
    w     = [w0_0, rot(theta_m) @ (a_m, b_m)] [B,L,D]   (w0 = S^-1 z0)
    out   = S @ w
Since cumsum commutes with the (linear) projection by om, we host-compute
cumx = cumsum(x, axis=1) ([B,L,2], tiny) and device-compute
    theta' = cumx @ (om/2pi)^T                 (in revolutions)
    s, c   = sin/cos(2pi theta')               (range-reduced on device)
    out    = c @ P^T + s @ Q^T + const         (P,Q fold a,b into S columns)
with P[i,m] = S[i,1+2m] a_m + S[i,2+2m] b_m, Q[i,m] = S[i,2+2m] a_m - S[i,1+2m] b_m,
const = S[:,0] w0_0 (added host-side during the gather).

Sharding: pure data-parallel, batches 4i..4i+3 on core i, no collectives.
Device matmuls run in bf16 (hi/lo split for the angle projection so theta'
keeps ~fp27 precision); accumulation is fp32 in PSUM.
"""
import math

import numpy as np
import ml_dtypes

import concourse.bacc as bacc
import concourse.mybir as mybir
from concourse.tile import TileContext
from concourse import bass_utils

B, L, D = 32, 2048, 513
M = (D - 1) // 2          # 256
NCORES = 8
BS = B // NCORES          # 4 batches per core
LG = 512                  # l-group width (per theta tile)
NG = BS * (L // LG)       # 16 l-groups per core
MAGIC = 12582912.0        # 1.5 * 2^23: fp32 round-to-nearest-int trick
TWO_PI = 2.0 * math.pi

F32 = mybir.dt.float32
BF16 = mybir.dt.bfloat16
ALU = mybir.AluOpType
ACTF = mybir.ActivationFunctionType


def _build():
    nc = bacc.Bacc("TRN2", target_bir_lowering=False, debug=False)

    cumx6_d = nc.dram_tensor("cumx6", [6, BS * L], BF16, kind="ExternalInput")
    th_lhsT_d = nc.dram_tensor("th_lhsT", [6, M], BF16, kind="ExternalInput")
    pqt_d = nc.dram_tensor("pqt", [128, 4 * D], BF16, kind="ExternalInput")
    out_d = nc.dram_tensor("out", [BS, L, D], F32, kind="ExternalOutput")

    with TileContext(nc) as tc:
        with (
            tc.tile_pool(name="const", bufs=1) as cpool,
            tc.tile_pool(name="work", bufs=2) as wpool,
            tc.tile_pool(name="cs", bufs=4) as cspool,
            tc.tile_pool(name="osb", bufs=3) as opool,
            tc.tile_pool(name="thp", bufs=2, space="PSUM") as thpool,
            tc.tile_pool(name="pa", bufs=3, space="PSUM") as papool,
            tc.tile_pool(name="pb", bufs=3, space="PSUM") as pbpool,
        ):
            cumx6 = cpool.tile([6, BS * L], BF16)
            nc.sync.dma_start(out=cumx6[:], in_=cumx6_d.ap())
            th_lhsT = cpool.tile([6, M], BF16)
            nc.sync.dma_start(out=th_lhsT[:], in_=th_lhsT_d.ap())
            pqt = cpool.tile([128, 4 * D], BF16)
            nc.sync.dma_start(out=pqt[:], in_=pqt_d.ap())

            for g in range(NG):
                b, l0 = divmod(g, L // LG)
                l0 *= LG
                cs = []  # [c0, c1, s0, s1] tiles, [m=128, l=512] bf16
                for mc in range(2):
                    th = thpool.tile([128, LG], F32)
                    nc.tensor.matmul(
                        th[:],
                        th_lhsT[:, mc * 128:(mc + 1) * 128],
                        cumx6[:, b * L + l0: b * L + l0 + LG],
                        start=True, stop=True,
                    )
                    k = wpool.tile([128, LG], F32, tag="k")
                    nc.vector.tensor_scalar(k[:], th[:], MAGIC, MAGIC, ALU.add, ALU.subtract)
                    r = wpool.tile([128, LG], F32, tag="r")
                    nc.vector.scalar_tensor_tensor(r[:], k[:], -1.0, th[:], ALU.mult, ALU.add)
                    rc = wpool.tile([128, LG], F32, tag="rc")
                    nc.vector.add_range_wrap(rc[:], r[:], shift=0.25, bound=0.5, period=1.0)
                    s_t = cspool.tile([128, LG], BF16, tag="s")
                    nc.scalar.activation(s_t[:], r[:], ACTF.Sin, scale=TWO_PI)
                    c_t = cspool.tile([128, LG], BF16, tag="c")
                    nc.scalar.activation(c_t[:], rc[:], ACTF.Sin, scale=TWO_PI)
                    cs.append((c_t, s_t))
                # K-chunk order must match pqt rows: [P m0, P m1, Q m0, Q m1]
                kchunks = [cs[0][0], cs[1][0], cs[0][1], cs[1][1]]

                o = opool.tile([128, 4 * D], F32)
                for lc in range(4):
                    pa = papool.tile([128, 256], F32)
                    pb = pbpool.tile([128, 257], F32)
                    for kc in range(4):
                        # stride-4 slice: psum partition p <-> l = l0 + 4p + lc,
                        # so each SBUF partition covers 4 consecutive DRAM rows
                        # and the output DMA is one sequential 1 MiB write.
                        lhsT = kchunks[kc][:, lc:LG:4]
                        nc.tensor.matmul(pa[:], lhsT, pqt[:, kc * D: kc * D + 256],
                                         start=(kc == 0), stop=(kc == 3))
                        nc.tensor.matmul(pb[:], lhsT, pqt[:, kc * D + 256: (kc + 1) * D],
                                         start=(kc == 0), stop=(kc == 3))
                    nc.any.tensor_copy(o[:, lc * D: lc * D + 256], pa[:])
                    nc.any.tensor_copy(o[:, lc * D + 256: (lc + 1) * D], pb[:])
                nc.sync.dma_start(
                    out=out_d.ap()[b, l0: l0 + LG, :].rearrange("(p j) i -> p j i", p=128),
                    in_=o[:].rearrange("p (j i) -> p j i", j=4),
                )
    nc.compile()
    return nc


_NC = None


def _get_nc():
    global _NC
    if _NC is None:
        _NC = _build()
    return _NC


def _bf16(x):
    return np.asarray(x, dtype=np.float32).astype(ml_dtypes.bfloat16)


def _prep(x, S, om, z0):
    """Host-side prep: exact cumsum + tiny dense algebra; returns per-core
    input maps plus the additive constant."""
    x = np.asarray(x, dtype=np.float32)
    S = np.asarray(S, dtype=np.float32)
    om = np.asarray(om, dtype=np.float32)
    z0 = np.asarray(z0, dtype=np.float32)

    S_inv = np.linalg.inv(S)
    w0 = S_inv @ z0
    a, bb = w0[1::2], w0[2::2]
    # P/Q: out[:, i] = sum_m c_m P[i,m] + s_m Q[i,m] + const_i
    Scol = S[:, 1:]                      # [D, 2M]
    P = Scol[:, 0::2] * a + Scol[:, 1::2] * bb     # [D, M]
    Q = Scol[:, 1::2] * a - Scol[:, 0::2] * bb
    const = (S[:, 0] * w0[0]).astype(np.float32)   # [D]
    pqt = np.concatenate([P.T, Q.T], axis=0)       # [2M=512, D]
    # device layout: [128 partitions, (kc, i)] with kc = K-chunk index
    pqt_bf = np.ascontiguousarray(
        _bf16(pqt).reshape(4, 128, D).transpose(1, 0, 2).reshape(128, 4 * D)
    )

    omr = (om / TWO_PI).astype(np.float32)         # [M, 2] in revolutions
    oh = _bf16(omr).astype(np.float32)
    ol = _bf16(omr - oh).astype(np.float32)
    th_lhsT = np.stack(
        [oh[:, 0], oh[:, 1], oh[:, 0], oh[:, 1], ol[:, 0], ol[:, 1]], axis=0
    )                                              # [6, M]
    th_lhsT_bf = _bf16(th_lhsT)

    cumx = np.cumsum(x, axis=1)                    # [B, L, 2] fp32
    ch = _bf16(cumx).astype(np.float32)
    cl = _bf16(cumx - ch).astype(np.float32)
    # rhs rows pair with th_lhsT rows: [chx, chy, clx, cly, chx, chy]
    cumx6 = np.stack(
        [ch[..., 0], ch[..., 1], cl[..., 0], cl[..., 1], ch[..., 0], ch[..., 1]],
        axis=1,
    )                                              # [B, 6, L]
    cumx6_bf = _bf16(cumx6)

    in_maps = []
    for ci in range(NCORES):
        shard = cumx6_bf[ci * BS:(ci + 1) * BS]            # [BS, 6, L]
        shard = np.ascontiguousarray(
            shard.transpose(1, 0, 2).reshape(6, BS * L)    # [6, (b l)]
        )
        in_maps.append({
            "cumx6": shard,
            "th_lhsT": th_lhsT_bf,
            "pqt": pqt_bf,
        })
    return in_maps, const


def kernel(x, S, om, z0):
    nc = _get_nc()
    in_maps, const = _prep(x, S, om, z0)
    res = bass_utils.run_bass_kernel_spmd(nc, in_maps, core_ids=list(range(NCORES)))
    out = np.concatenate([res.results[i]["out"] for i in range(NCORES)], axis=0)
    out += const[None, None, :]
    z_n = np.ascontiguousarray(out[:, -1])
    return out, z_n
